# revision 7
# baseline (speedup 1.0000x reference)
"""Leaky RNN layer on 8 Trainium2 NeuronCores (Bass/Tile).

  h_{t+1} = (1-a)*o_t + a*(x_t W_in + h_t W_hidden + sigma_t),  o_{t+1} = tanh(h_{t+1})
  out[:, 0, :] = 0, out[:, t+1, :] = o_{t+1}, plus final pre-activation state h_final.

Sharding: data-parallel over batch (128 -> 8 cores x 16 rows), weights replicated.
Per core, per step, everything is accumulated in one PSUM tile [16, 512]:
  psum = a*x_t@W_in  (lhsT = xT columns for step t)
       + a*sigma_t   (lhsT = a*I16, rhs = sigma ring slot)
       + (1-a)*o_t   (lhsT = (1-a)*I16, rhs = o ring slot)
       + h_t@(a*W)   (4 k-tile matmuls, lhsT = transposed state hT)
then tanh(psum) -> o ring, and psum is copied + PE-transposed back into hT for
the next step. Sigma streams in / o streams out through SBUF rings in chunks.
"""

import numpy as np

B, T, I, H = 128, 1024, 64, 512
NCORES = 8
BL = B // NCORES        # 16 batch rows per core
ALPHA = 0.1             # DT / TAU
LEAK = 1.0 - ALPHA
CH = 8                  # timesteps per chunk (DMA granularity)
RING = 16               # ring slots (2 chunks)
KT = 4                  # k-tiles of the hidden contraction (512 / 128)

_CACHE = {}


def _build(t_len):
    import concourse.bass as bass
    import concourse.tile as tile
    from concourse import bacc, mybir

    dt = mybir.dt.float32
    nsteps = t_len - 1
    nchunks = (nsteps + CH - 1) // CH

    nc = bacc.Bacc("TRN2", debug=False)
    xT = nc.declare_dram_parameter("xT", [I, t_len * BL], dt, isOutput=False)
    sig = nc.declare_dram_parameter("sig", [BL, t_len, H], dt, isOutput=False)
    w_in = nc.declare_dram_parameter("w_in", [I, H], dt, isOutput=False)
    w_h = nc.declare_dram_parameter("w_h", [128, KT * H], dt, isOutput=False)
    oi = nc.declare_dram_parameter("oi", [BL, BL], dt, isOutput=False)
    ai = nc.declare_dram_parameter("ai", [BL, BL], dt, isOutput=False)
    ident = nc.declare_dram_parameter("ident", [BL, BL], dt, isOutput=False)
    h0T = nc.declare_dram_parameter("h0T", [128, KT * BL], dt, isOutput=False)
    out = nc.declare_dram_parameter("out", [BL, t_len, H], dt, isOutput=True)
    hfin = nc.declare_dram_parameter("hfin", [BL, H], dt, isOutput=True)

    tanh = mybir.ActivationFunctionType.Tanh

    with tile.TileContext(nc) as tc:
        with (
            tc.tile_pool(name="const", bufs=1) as constp,
            tc.tile_pool(name="state", bufs=2) as statep,
            tc.tile_pool(name="hsb", bufs=2) as hsbp,
            tc.tile_pool(name="ph", bufs=2, space="PSUM") as php,
            tc.tile_pool(name="pt", bufs=2, space="PSUM") as ptp,
            tc.tile_pool(name="touch", bufs=1, space="PSUM") as touchp,
        ):
            xT_sb = constp.tile([I, t_len * BL], dt)
            w_in_sb = constp.tile([I, H], dt)
            w_h_sb = constp.tile([128, KT * H], dt)
            oi_sb = constp.tile([BL, BL], dt)
            ai_sb = constp.tile([BL, BL], dt)
            id_sb = constp.tile([BL, BL], dt)
            ring_s = constp.tile([BL, RING, H], dt)
            ring_o = constp.tile([BL, RING, H], dt)

            nc.sync.dma_start(xT_sb[:, :], xT[:, :])
            nc.sync.dma_start(w_in_sb[:, :], w_in[:, :])
            nc.sync.dma_start(w_h_sb[:, :], w_h[:, :])
            nc.sync.dma_start(oi_sb[:, :], oi[:, :])
            nc.sync.dma_start(ai_sb[:, :], ai[:, :])
            nc.sync.dma_start(id_sb[:, :], ident[:, :])

            hT_prev = statep.tile([128, KT * BL], dt, name="hT")
            nc.sync.dma_start(hT_prev[:, :], h0T[:, :])

            # o_0 = 0; also emit out[:, 0, :] = 0 from the same zero slot.
            nc.vector.memset(ring_o[:, 0, :], 0.0)
            nc.sync.dma_start(out[:, 0, :], ring_o[:, 0, :])

            def sig_prefetch(q):
                t0 = q * CH
                n = min(CH, nsteps - t0)
                if n <= 0:
                    return
                s0 = t0 % RING
                nc.sync.dma_start(ring_s[:, s0:s0 + n, :], sig[:, t0:t0 + n, :])

            sig_prefetch(0)
            sig_prefetch(1)

            # PE "touch" of every DMA-loaded tile: walrus allows only one
            # sync-wait on a Matmult's LDWEIGHTS, so advance the PE's
            # observed clock one semaphore at a time before the real
            # matmuls (which would otherwise accumulate several waits).
            touch_ps = touchp.tile([1, 16], dt, name="touch_ps")
            touch_sb = constp.tile([1, 16], dt, name="touch_sb")
            touched = [xT_sb, w_in_sb, w_h_sb, oi_sb, ai_sb, id_sb, hT_prev]
            for idx, tl in enumerate(touched):
                nc.tensor.transpose(touch_ps[0:1, idx:idx + 1],
                                    tl[:, 0:1], tl[:, 0:1])
                nc.vector.tensor_copy(touch_sb[0:1, idx:idx + 1], tl[0:1, 0:1])
            nc.tensor.transpose(touch_ps[0:1, len(touched):len(touched) + 1],
                                ring_s[:, 0, 0:1], ring_s[:, 0, 0:1])
            nc.vector.tensor_copy(touch_sb[0:1, len(touched):len(touched) + 1],
                                  ring_s[0:1, 0, 0:1])

            h_sb = None
            for q in range(nchunks):
                t0 = q * CH
                n = min(CH, nsteps - t0)
                for s in range(n):
                    t = t0 + s
                    slot = t % RING
                    ph = php.tile([BL, H], dt, name="ph")
                    nc.tensor.matmul(ph[:, :], xT_sb[:, t * BL:(t + 1) * BL],
                                     w_in_sb[:, :], start=True, stop=False)
                    nc.tensor.matmul(ph[:, :], ai_sb[:, :], ring_s[:, slot, :],
                                     start=False, stop=False)
                    nc.tensor.matmul(ph[:, :], oi_sb[:, :], ring_o[:, slot, :],
                                     start=False, stop=False)
                    for k in range(KT):
                        nc.tensor.matmul(ph[:, :],
                                         hT_prev[:, k * BL:(k + 1) * BL],
                                         w_h_sb[:, k * H:(k + 1) * H],
                                         start=False, stop=(k == KT - 1))
                    nc.scalar.activation(ring_o[:, (t + 1) % RING, :], ph[:, :], tanh)
                    h_sb = hsbp.tile([BL, H], dt, name="h_sb")
                    nc.vector.tensor_copy(h_sb[:, :], ph[:, :])
                    if t < nsteps - 1:
                        pt = ptp.tile([128, KT * BL], dt, name="pt")
                        for j in range(KT):
                            nc.tensor.transpose(pt[:, j * BL:(j + 1) * BL],
                                                h_sb[:, j * 128:(j + 1) * 128],
                                                id_sb[:, :])
                        hT_prev = statep.tile([128, KT * BL], dt, name="hT")
                        nc.vector.tensor_copy(hT_prev[:, :], pt[:, :])

                # store o_{t0+1} .. o_{t0+n} -> out[:, t0+1 : t0+n+1, :]
                s_start = (t0 + 1) % RING
                n1 = min(n, RING - s_start)
                nc.sync.dma_start(out[:, t0 + 1:t0 + 1 + n1, :],
                                  ring_o[:, s_start:s_start + n1, :])
                if n1 < n:
                    nc.sync.dma_start(out[:, t0 + 1 + n1:t0 + 1 + n, :],
                                      ring_o[:, 0:n - n1, :])
                sig_prefetch(q + 2)

            nc.sync.dma_start(hfin[:, :], h_sb[:, :])

    nc.finalize()
    return nc


def _prep_maps(x, sigma, W_in, W_hidden, h0, t_len):
    aW_in = np.ascontiguousarray(ALPHA * W_in).astype(np.float32)
    aW_h = (ALPHA * W_hidden).astype(np.float32)
    w_h_tiled = np.ascontiguousarray(
        aW_h.reshape(KT, 128, H).transpose(1, 0, 2).reshape(128, KT * H))
    oi = LEAK * np.eye(BL, dtype=np.float32)
    ai = ALPHA * np.eye(BL, dtype=np.float32)
    ident = np.eye(BL, dtype=np.float32)
    in_maps = []
    for c in range(NCORES):
        rows = slice(c * BL, (c + 1) * BL)
        xc = np.asarray(x[rows], dtype=np.float32)          # [16, T, 64]
        xT = np.ascontiguousarray(xc.transpose(2, 1, 0).reshape(I, t_len * BL))
        h0c = np.asarray(h0[rows], dtype=np.float32)        # [16, 512]
        h0T = np.ascontiguousarray(
            h0c.T.reshape(KT, 128, BL).transpose(1, 0, 2).reshape(128, KT * BL))
        in_maps.append({
            "xT": xT,
            "sig": np.ascontiguousarray(sigma[rows]).astype(np.float32),
            "w_in": aW_in, "w_h": w_h_tiled,
            "oi": oi, "ai": ai, "ident": ident, "h0T": h0T,
        })
    return in_maps


def _run(x, sigma, W_in, W_hidden, h0, trace=False):
    from concourse.bass_utils import run_bass_kernel_spmd

    t_len = x.shape[1]
    if t_len not in _CACHE:
        _CACHE[t_len] = _build(t_len)
    nc = _CACHE[t_len]
    in_maps = _prep_maps(x, sigma, W_in, W_hidden, h0, t_len)
    res = run_bass_kernel_spmd(nc, in_maps, list(range(NCORES)), trace=trace)
    bsz = x.shape[0]
    outs = np.empty((bsz, t_len, H), np.float32)
    hfin = np.empty((bsz, H), np.float32)
    for c in range(NCORES):
        rows = slice(c * BL, (c + 1) * BL)
        outs[rows] = res.results[c]["out"]
        hfin[rows] = res.results[c]["hfin"]
    return (outs, hfin), res


def kernel(x, sigma, W_in, W_hidden, h0):
    (outs, hfin), _ = _run(x, sigma, W_in, W_hidden, h0)
    return outs, hfin


# revision 18
# speedup vs baseline: 5.9061x; 5.9061x over previous
"""Leaky RNN layer on 8 Trainium2 NeuronCores (Bass/Tile).

  h_{t+1} = (1-a)*o_t + a*(x_t W_in + h_t W_hidden + sigma_t),  o_{t+1} = tanh(h_{t+1})
  out[:, 0, :] = 0, out[:, t+1, :] = o_{t+1}, plus final pre-activation state h_final.

Sharding: data-parallel over batch (128 -> 8 cores x 16 rows), weights replicated.

Per core the recurrent state is kept TRANSPOSED as hT [128, 4, 16] with
hT[p, k, b] = h[b, 128k+p], so each step is 16 small fp16 matmuls
  ph[:, j, :]  +=  W[ktile, jtile].T @ hT[:, k, :]     (N = 16)
whose output is already in hT layout -- no per-step transposes.  The
z_t = a*(x_t W_in + sigma_t) term is precomputed per 8-step chunk in the
natural [t*b, h] orientation (2 fp16 matmuls into PSUM), transposed on
the PE, and parked in an SBUF ring; the leaky (1-a)*o_t term is folded
with z_t into u_t on the DVE off the critical path.  The per-step chain
is just: 16 matmuls -> one DVE add [128, 64] -> next step.

Outputs stream to DRAM in transposed layout and are unscrambled on host.
"""

import numpy as np

B, T, I, H = 128, 1024, 64, 512
NCORES = 8
BL = B // NCORES        # 16 batch rows per core
ALPHA = 0.1             # DT / TAU
LEAK = 1.0 - ALPHA
CH = 8                  # timesteps per chunk (DMA + phase-1 granularity)
RING = 16               # ring slots (2 chunks)
KT = 4                  # k/j tiles of the hidden dim (512 / 128)

_CACHE = {}


def _build(t_len):
    import concourse.bass as bass
    import concourse.tile as tile
    from concourse import bacc, mybir

    f32 = mybir.dt.float32
    f16 = mybir.dt.float16
    nsteps = t_len - 1
    nchunks = (nsteps + CH - 1) // CH

    nc = bacc.Bacc("TRN2", debug=False)
    xT = nc.declare_dram_parameter("xT", [I, t_len * BL], f16, isOutput=False)
    sig = nc.declare_dram_parameter("sig", [t_len * BL, H], f16, isOutput=False)
    w_in = nc.declare_dram_parameter("w_in", [I, H], f16, isOutput=False)     # alpha*W_in
    w_h = nc.declare_dram_parameter("w_h", [128, KT * KT * 128], f16, isOutput=False)
    ai = nc.declare_dram_parameter("ai", [128, 128], f16, isOutput=False)     # alpha*I128
    ident = nc.declare_dram_parameter("ident", [128, 128], f16, isOutput=False)
    h0T = nc.declare_dram_parameter("h0T", [128, KT * BL], f16, isOutput=False)
    oscr = nc.declare_dram_parameter("oscr", [t_len, 128, KT, BL], f32, isOutput=True)
    hfinT = nc.declare_dram_parameter("hfinT", [128, KT * BL], f32, isOutput=True)

    tanh = mybir.ActivationFunctionType.Tanh
    mult = mybir.AluOpType.mult
    add = mybir.AluOpType.add

    with tile.TileContext(nc) as tc:
        with (
            tc.tile_pool(name="const", bufs=1) as constp,
            tc.tile_pool(name="state", bufs=2) as statep,
            tc.tile_pool(name="upool", bufs=2) as upool,
            tc.tile_pool(name="sigp", bufs=2) as sigp,
            tc.tile_pool(name="zsbp", bufs=2) as zsbp,
            tc.tile_pool(name="pz", bufs=2, space="PSUM") as pzp,
            tc.tile_pool(name="pzt", bufs=2, space="PSUM") as pztp,
            tc.tile_pool(name="ph", bufs=2, space="PSUM") as php,
            tc.tile_pool(name="touch", bufs=1, space="PSUM") as touchp,
        ):
            xT_sb = constp.tile([I, t_len * BL], f16)
            w_in_sb = constp.tile([I, H], f16)
            w_h_sb = constp.tile([128, KT * KT * 128], f16)
            ai_sb = constp.tile([128, 128], f16)
            id_sb = constp.tile([128, 128], f16)
            ring_z = constp.tile([128, RING, KT, BL], f32)
            ring_o = constp.tile([128, RING, KT, BL], f32)

            nc.sync.dma_start(xT_sb[:, :], xT[:, :])
            nc.sync.dma_start(w_in_sb[:, :], w_in[:, :])
            nc.sync.dma_start(w_h_sb[:, :], w_h[:, :])
            nc.sync.dma_start(ai_sb[:, :], ai[:, :])
            nc.sync.dma_start(id_sb[:, :], ident[:, :])

            hT_prev = statep.tile([128, KT, BL], f16, name="hT")
            nc.sync.dma_start(
                hT_prev[:, :, :],
                h0T[:, :].rearrange("p (k b) -> p k b", k=KT))

            # o_0 = 0 (both the ring slot and out[:, 0, :]).
            nc.vector.memset(ring_o[:, 0, :, :], 0.0)
            nc.sync.dma_start(
                oscr[0, :, :, :], ring_o[:, 0, :, :])

            # One PE / DVE touch per DMA-loaded tile: walrus allows only one
            # sync-wait per instruction, so advance each engine's observed
            # clock one semaphore at a time before the real work.
            # id_sb first so later touches can use it as the permutation rhs.
            touch_ps = touchp.tile([1, 6 * 128], f16, name="touch_ps")
            touch_sb = constp.tile([1, 16], f32, name="touch_sb")
            touched = [id_sb, xT_sb, w_in_sb, w_h_sb, ai_sb]
            for idx, tl in enumerate(touched):
                kp = tl.shape[0]
                nc.tensor.transpose(touch_ps[0:1, idx * 128:idx * 128 + kp],
                                    tl[:, 0:1], id_sb[0:kp, 0:kp])
                nc.vector.tensor_copy(touch_sb[0:1, idx:idx + 1], tl[0:1, 0:1])
            nc.tensor.transpose(touch_ps[0:1, 5 * 128:6 * 128],
                                hT_prev[:, 0, 0:1], id_sb[:, :])
            nc.vector.tensor_copy(touch_sb[0:1, 5:6], hT_prev[0:1, 0, 0:1])

            def phase1(q):
                """Produce ring_z slots for chunk q (z_t, transposed)."""
                t0 = q * CH
                n = min(CH, nsteps - t0)
                if n <= 0:
                    return
                nb = n * BL
                sig_sb = sigp.tile([128, H], f16, name="sig_sb")
                nc.sync.dma_start(
                    sig_sb[0:nb, :], sig[t0 * BL:t0 * BL + nb, :])
                pz = pzp.tile([128, H], f32, name="pz")
                nc.tensor.matmul(pz[0:nb, :], xT_sb[:, t0 * BL:t0 * BL + nb],
                                 w_in_sb[:, :], start=True, stop=False)
                nc.tensor.matmul(pz[0:nb, :], ai_sb[0:nb, 0:nb],
                                 sig_sb[0:nb, :], start=False, stop=True)
                z_sb = zsbp.tile([128, H], f16, name="z_sb")
                nc.vector.tensor_copy(z_sb[0:nb, :], pz[0:nb, :])
                pzt = pztp.tile([128, KT, CH, BL], f16, name="pzt")
                for j in range(KT):
                    nc.tensor.transpose(
                        pzt[:, j, 0:n, :],
                        z_sb[0:nb, j * 128:(j + 1) * 128], id_sb[0:nb, 0:nb])
                s0 = t0 % RING
                for j in range(KT):
                    nc.vector.tensor_copy(
                        ring_z[:, s0:s0 + n, j, :], pzt[:, j, 0:n, :])

            phase1(0)
            phase1(1)

            hfin_sb = None
            for q in range(nchunks):
                t0 = q * CH
                n = min(CH, nsteps - t0)
                for s in range(n):
                    t = t0 + s
                    slot = t % RING
                    last = (t == nsteps - 1)

                    # u_t = z_t + (1-a)*o_t   (transposed layout, off-chain)
                    u_t = upool.tile([128, KT, BL], f32, name="u_t")
                    nc.vector.scalar_tensor_tensor(
                        u_t[:, :, :], ring_o[:, slot, :, :], LEAK,
                        ring_z[:, slot, :, :], mult, add)

                    ph = php.tile([128, KT, BL], f32, name="ph")
                    for j in range(KT):
                        for k in range(KT):
                            nc.tensor.matmul(
                                ph[:, j, :],
                                w_h_sb[:, (k * KT + j) * 128:(k * KT + j + 1) * 128],
                                hT_prev[:, k, :],
                                start=(k == 0), stop=(k == KT - 1))

                    if not last:
                        hT_next = statep.tile([128, KT, BL], f16, name="hT")
                        nc.vector.tensor_add(hT_next[:, :, :], ph[:, :, :],
                                             u_t[:, :, :])
                        nc.scalar.activation(ring_o[:, (t + 1) % RING, :, :],
                                             hT_next[:, :, :], tanh)
                        hT_prev = hT_next
                    else:
                        hfin_sb = statep.tile([128, KT, BL], f32, name="hT_fin")
                        nc.vector.tensor_add(hfin_sb[:, :, :], ph[:, :, :],
                                             u_t[:, :, :])
                        nc.scalar.activation(ring_o[:, (t + 1) % RING, :, :],
                                             hfin_sb[:, :, :], tanh)

                # store o_{t0+1} .. o_{t0+n} -> oscr[t0+1 : t0+n+1]
                s_start = (t0 + 1) % RING
                n1 = min(n, RING - s_start)
                nc.sync.dma_start(
                    oscr[t0 + 1:t0 + 1 + n1, :, :, :].rearrange("t p j b -> p t j b"),
                    ring_o[:, s_start:s_start + n1, :, :])
                if n1 < n:
                    nc.sync.dma_start(
                        oscr[t0 + 1 + n1:t0 + 1 + n, :, :, :].rearrange("t p j b -> p t j b"),
                        ring_o[:, 0:n - n1, :, :])
                phase1(q + 2)

            nc.sync.dma_start(
                hfinT[:, :].rearrange("p (k b) -> p k b", k=KT),
                hfin_sb[:, :, :])

    nc.finalize()
    return nc


def _prep_maps(x, sigma, W_in, W_hidden, h0, t_len):
    aW_in = np.ascontiguousarray(ALPHA * np.asarray(W_in)).astype(np.float16)
    aW_h = (ALPHA * np.asarray(W_hidden)).astype(np.float32)
    # w_h_sb[p, (k*KT+j)*128 + m] = aW_h[128k+p, 128j+m]
    w_h_tiled = np.ascontiguousarray(
        aW_h.reshape(KT, 128, KT, 128).transpose(1, 0, 2, 3)
        .reshape(128, KT * KT * 128)).astype(np.float16)
    ai = (ALPHA * np.eye(128)).astype(np.float16)
    ident = np.eye(128, dtype=np.float16)
    in_maps = []
    for c in range(NCORES):
        rows = slice(c * BL, (c + 1) * BL)
        xc = np.asarray(x[rows], dtype=np.float32)          # [16, T, 64]
        xTc = np.ascontiguousarray(
            xc.transpose(2, 1, 0).reshape(I, t_len * BL)).astype(np.float16)
        h0c = np.asarray(h0[rows], dtype=np.float32)        # [16, 512]
        h0T = np.ascontiguousarray(
            h0c.T.reshape(KT, 128, BL).transpose(1, 0, 2)
            .reshape(128, KT * BL)).astype(np.float16)
        sc = np.asarray(sigma[rows], dtype=np.float32)      # [16, T, 512]
        sigc = np.ascontiguousarray(
            sc.transpose(1, 0, 2).reshape(t_len * BL, H)).astype(np.float16)
        in_maps.append({
            "xT": xTc,
            "sig": sigc,
            "w_in": aW_in, "w_h": w_h_tiled,
            "ai": ai, "ident": ident, "h0T": h0T,
        })
    return in_maps


def _run(x, sigma, W_in, W_hidden, h0, trace=False):
    from concourse.bass_utils import run_bass_kernel_spmd

    t_len = x.shape[1]
    if t_len not in _CACHE:
        _CACHE[t_len] = _build(t_len)
    nc = _CACHE[t_len]
    in_maps = _prep_maps(x, sigma, W_in, W_hidden, h0, t_len)
    res = run_bass_kernel_spmd(nc, in_maps, list(range(NCORES)), trace=trace)
    bsz = x.shape[0]
    outs = np.empty((bsz, t_len, H), np.float32)
    hfin = np.empty((bsz, H), np.float32)
    for c in range(NCORES):
        rows = slice(c * BL, (c + 1) * BL)
        o = np.asarray(res.results[c]["oscr"])      # [T, 128, KT, BL]
        outs[rows] = o.transpose(3, 0, 2, 1).reshape(BL, t_len, H)
        hf = np.asarray(res.results[c]["hfinT"])    # [128, KT*BL]
        hfin[rows] = (hf.reshape(128, KT, BL)
                      .transpose(2, 1, 0).reshape(BL, H))
    return (outs, hfin), res


def kernel(x, sigma, W_in, W_hidden, h0):
    (outs, hfin), _ = _run(x, sigma, W_in, W_hidden, h0)
    return outs, hfin
